# revision 1
# baseline (speedup 1.0000x reference)
"""Trainium2 Bass kernel for nn_MultiHeadDistanceLayer (sparse_attention).

Math: for each (head h, batch b) the reference collapses to
    S[m] = sum_k attn[k-m, k] * w[k],   w[k] = sigmoid(x @ Wv)[L-1-k, h]
(weighted superdiagonal sums of the attention matrix), followed by a
window-3 same-padded average pool over m (padding excluded from the
divisor):  out[b, m, h] = (S[m-1] + S[m] + S[m+1]) / cnt[m].

Sharding: 8 heads -> 8 NeuronCores; each core computes its head for both
batches.  Per (h, b) the kernel runs flash-style over 16 row blocks of 128
queries: scores via PE (K=HD=32), exp on ACT (with free row-sum accum ->
softmax denominators d), gating by w on DVE, then a *skewed* DMA write of
the probability block to a DRAM scratch so that superdiagonal m lands at
column m of every row.  A plain strided read back + a [1/d]-weighted
ones-matmul on PE reduces partitions, PSUM-accumulating S[m] across
blocks.  The tiny pool runs on-device; host only slices inputs / stacks
the (B, L) per-core outputs into (B, L, H).
"""

import numpy as np

import concourse.bacc as bacc
import concourse.bass as bass
import concourse.tile as tile
from concourse import mybir
from concourse.bass_utils import run_bass_kernel_spmd
from concourse.tile import add_dep_helper

B, L, D, H, HD, WIN = 2, 2048, 128, 8, 32, 3
NBLK = L // 128           # 16 row blocks per (h, b)
W = L + 128               # scratch row width (elements)
SCALE = float(HD) ** -0.5

FP16 = mybir.dt.float16
FP32 = mybir.dt.float32


def build_nc():
    nc = bacc.Bacc("TRN2", target_bir_lowering=False, debug=False)

    xpT = nc.dram_tensor("xpT", [B, D, L], FP16, kind="ExternalInput")
    xrevT = nc.dram_tensor("xrevT", [B, D, L], FP16, kind="ExternalInput")
    wq = nc.dram_tensor("wq", [D, HD], FP16, kind="ExternalInput")
    wk = nc.dram_tensor("wk", [D, HD], FP16, kind="ExternalInput")
    wv = nc.dram_tensor("wv", [D, 1], FP16, kind="ExternalInput")
    bq = nc.dram_tensor("bq", [HD, 1], FP32, kind="ExternalInput")
    bk = nc.dram_tensor("bk", [HD, 1], FP32, kind="ExternalInput")
    out = nc.dram_tensor("out", [B, L], FP32, kind="ExternalOutput")
    # one flat scratch region per (h, b) pair; row i of the logical [L, W]
    # grid holds the skew-shifted probability row i
    scr = [
        nc.dram_tensor(f"scr{b}", [L * W], FP16, kind="Internal") for b in range(B)
    ]

    with tile.TileContext(nc) as tc:
        import contextlib

        with contextlib.ExitStack() as ctx:
            singles = ctx.enter_context(tc.tile_pool(name="singles", bufs=1))
            small = ctx.enter_context(tc.tile_pool(name="small", bufs=4))
            ew_pool = ctx.enter_context(tc.tile_pool(name="ew", bufs=3))
            rt_pool = ctx.enter_context(tc.tile_pool(name="rt", bufs=3))
            ps_pool = ctx.enter_context(tc.tile_pool(name="ps", bufs=2, space="PSUM"))
            s_pool = ctx.enter_context(tc.tile_pool(name="spsum", bufs=1, space="PSUM"))
            post_pool = ctx.enter_context(tc.tile_pool(name="post", bufs=2))

            # ---- constants ----
            ones_row = singles.tile([1, 128], FP16)
            nc.vector.memset(ones_row, 1.0)
            cntr = singles.tile([1, L], FP32)
            nc.vector.memset(cntr, 1.0 / 3.0)
            nc.vector.memset(cntr[0:1, 0:1], 0.5)
            nc.vector.memset(cntr[0:1, L - 1 : L], 0.5)

            bq_sb = singles.tile([HD, 1], FP32)
            nc.sync.dma_start(out=bq_sb, in_=bq[:, :])
            bk_sb = singles.tile([HD, 1], FP32)
            nc.sync.dma_start(out=bk_sb, in_=bk[:, :])
            wq_sb = singles.tile([D, HD], FP16)
            nc.sync.dma_start(out=wq_sb, in_=wq[:, :])
            wk_sb = singles.tile([D, HD], FP16)
            nc.sync.dma_start(out=wk_sb, in_=wk[:, :])
            wv_sb = singles.tile([D, 1], FP16)
            nc.sync.dma_start(out=wv_sb, in_=wv[:, :])

            # ---- per-pair prep: QT/KT [HD, L] fp16, w_bcast [128, L] fp16 ----
            xp_sb, QT, KT, WB = [], [], [], []
            for b in range(B):
                xp_t = singles.tile([D, L], FP16, name=f"xpT{b}")
                nc.sync.dma_start(out=xp_t, in_=xpT[b, :, :])
                xp_sb.append(xp_t)
                xr_t = singles.tile([D, L], FP16, name=f"xrevT{b}")
                nc.sync.dma_start(out=xr_t, in_=xrevT[b, :, :])

                qt = singles.tile([HD, L], FP16, name=f"QT{b}")
                kt = singles.tile([HD, L], FP16, name=f"KT{b}")
                for half in range(2):
                    c0 = half * 1024
                    pq = ps_pool.tile([128, 1024], FP32, tag="ps")
                    for j in range(2):
                        nc.tensor.matmul(
                            out=pq[:HD, j * 512 : (j + 1) * 512],
                            lhsT=wq_sb[:, :],
                            rhs=xp_t[:, c0 + j * 512 : c0 + (j + 1) * 512],
                            start=True,
                            stop=True,
                        )
                    nc.vector.tensor_scalar_add(
                        out=qt[:, c0 : c0 + 1024],
                        in0=pq[:HD, :],
                        scalar1=bq_sb,
                    )
                    pk = ps_pool.tile([128, 1024], FP32, tag="ps")
                    for j in range(2):
                        nc.tensor.matmul(
                            out=pk[:HD, j * 512 : (j + 1) * 512],
                            lhsT=wk_sb[:, :],
                            rhs=xp_t[:, c0 + j * 512 : c0 + (j + 1) * 512],
                            start=True,
                            stop=True,
                        )
                    nc.vector.tensor_scalar_add(
                        out=kt[:, c0 : c0 + 1024],
                        in0=pk[:HD, :],
                        scalar1=bk_sb,
                    )
                QT.append(qt)
                KT.append(kt)

                # reversed gate: vrev[k] = sigmoid((x_rev @ Wv))[k] = v[L-1-k]
                vrev = small.tile([1, L], FP16, tag="vrev")
                for half in range(2):
                    c0 = half * 1024
                    pv = ps_pool.tile([128, 1024], FP32, tag="ps")
                    for j in range(2):
                        nc.tensor.matmul(
                            out=pv[0:1, j * 512 : (j + 1) * 512],
                            lhsT=wv_sb[:, :],
                            rhs=xr_t[:, c0 + j * 512 : c0 + (j + 1) * 512],
                            start=True,
                            stop=True,
                        )
                    nc.scalar.activation(
                        out=vrev[0:1, c0 : c0 + 1024],
                        in_=pv[0:1, :],
                        func=mybir.ActivationFunctionType.Sigmoid,
                    )
                # broadcast vrev across 128 partitions via K=1 matmul
                wb = singles.tile([128, L], FP16, name=f"WB{b}")
                for j in range(4):
                    pb = ps_pool.tile([128, 1024], FP32, tag="ps")
                    nc.tensor.matmul(
                        out=pb[:, 0:512],
                        lhsT=ones_row[:, :],
                        rhs=vrev[0:1, j * 512 : (j + 1) * 512],
                        start=True,
                        stop=True,
                    )
                    nc.vector.tensor_copy(
                        out=wb[:, j * 512 : (j + 1) * 512], in_=pb[:, 0:512]
                    )
                WB.append(wb)

            # ---- main per-(h,b) block pipeline ----
            for b in range(B):
                s_acc = s_pool.tile([1, L], FP32, tag="S")
                for r in range(NBLK):
                    i0 = r * 128
                    ew = ew_pool.tile([128, W], FP16, tag="ew")
                    nc.gpsimd.memset(ew[:, L:W], 0.0)
                    dcol = []
                    for half in range(2):
                        c0 = half * 1024
                        ps = ps_pool.tile([128, 1024], FP32, tag="ps")
                        for j in range(2):
                            nc.tensor.matmul(
                                out=ps[:, j * 512 : (j + 1) * 512],
                                lhsT=QT[b][:, i0 : i0 + 128],
                                rhs=KT[b][:, c0 + j * 512 : c0 + (j + 1) * 512],
                                start=True,
                                stop=True,
                            )
                        dc = small.tile([128, 1], FP32, tag="dc")
                        nc.scalar.activation(
                            out=ew[:, c0 : c0 + 1024],
                            in_=ps[:, :],
                            func=mybir.ActivationFunctionType.Exp,
                            scale=SCALE,
                            accum_out=dc,
                        )
                        dcol.append(dc)
                    dsum = small.tile([128, 1], FP32, tag="ds")
                    nc.vector.tensor_add(out=dsum, in0=dcol[0], in1=dcol[1])
                    drecip = small.tile([128, 1], FP32, tag="dr")
                    nc.vector.reciprocal(out=drecip, in_=dsum)
                    drecip16 = small.tile([128, 1], FP16, tag="dr16")
                    nc.vector.tensor_copy(out=drecip16, in_=drecip)

                    # gate by w (columns i0..L of this block are the upper tri)
                    nc.vector.tensor_mul(
                        out=ew[:, i0:L],
                        in0=ew[:, i0:L],
                        in1=WB[b][:, i0:L],
                    )

                    # skewed write: scratch[i0+p, c - i0 - p] = ew[p, c]
                    # flat dst = i0*W + p*(W-1) + (c - i0)
                    wlen = (L + 127) - i0
                    dst = bass.AP(
                        tensor=scr[b],
                        offset=i0 * W,
                        ap=[[W - 1, 128], [1, wlen]],
                    )
                    wr = nc.sync.dma_start(out=dst, in_=ew[:, i0 : L + 127])

                    # plain read back: rows i0..i0+128, cols 0..L-i0
                    rlen = L - i0
                    src = bass.AP(
                        tensor=scr[b],
                        offset=i0 * W,
                        ap=[[W, 128], [1, rlen]],
                    )
                    rt = rt_pool.tile([128, L], FP16, tag="rt")
                    rd = nc.sync.dma_start(out=rt[:, 0:rlen], in_=src)
                    add_dep_helper(rd.ins, wr.ins, True, "scratch RAW")

                    # S[m] += sum_p (1/d[i0+p]) * rt[p, m]
                    for j in range(4):
                        m0 = j * 512
                        wj = min(512, rlen - m0)
                        if wj <= 0:
                            continue
                        nc.tensor.matmul(
                            out=s_acc[0:1, m0 : m0 + wj],
                            lhsT=drecip16[:, 0:1],
                            rhs=rt[:, m0 : m0 + wj],
                            start=(r == 0),
                            stop=(r == 15 - 4 * j),
                        )

                # ---- pooling + output ----
                sS = post_pool.tile([1, L + 2], FP32, tag="sS")
                nc.vector.memset(sS[0:1, 0:1], 0.0)
                nc.vector.memset(sS[0:1, L + 1 : L + 2], 0.0)
                nc.vector.tensor_copy(out=sS[0:1, 1 : L + 1], in_=s_acc[0:1, :])
                t1 = post_pool.tile([1, L], FP32, tag="t1")
                nc.vector.tensor_add(
                    out=t1, in0=sS[0:1, 0:L], in1=sS[0:1, 1 : L + 1]
                )
                res = post_pool.tile([1, L], FP32, tag="res")
                nc.vector.tensor_add(out=res, in0=t1, in1=sS[0:1, 2 : L + 2])
                nc.vector.tensor_mul(out=res, in0=res, in1=cntr)
                nc.sync.dma_start(out=out[b, :], in_=res[0:1, :])

    nc.finalize()
    return nc


_NC = None


def _get_nc():
    global _NC
    if _NC is None:
        _NC = build_nc()
    return _NC


def kernel(x, pe, Wq, bq, Wk, bk, Wv):
    x = np.asarray(x, np.float32)
    pe = np.asarray(pe, np.float32)
    Wq = np.asarray(Wq, np.float32)
    bq = np.asarray(bq, np.float32)
    Wk = np.asarray(Wk, np.float32)
    bk = np.asarray(bk, np.float32)
    Wv = np.asarray(Wv, np.float32)

    xp = x + pe[None, :, :]
    xpT = np.ascontiguousarray(xp.transpose(0, 2, 1)).astype(np.float16)
    xrevT = np.ascontiguousarray(x[:, ::-1, :].transpose(0, 2, 1)).astype(np.float16)

    in_maps = []
    for h in range(H):
        in_maps.append(
            {
                "xpT": xpT,
                "xrevT": xrevT,
                "wq": np.ascontiguousarray(Wq[:, h * HD : (h + 1) * HD]).astype(
                    np.float16
                ),
                "wk": np.ascontiguousarray(Wk[:, h * HD : (h + 1) * HD]).astype(
                    np.float16
                ),
                "wv": np.ascontiguousarray(Wv[:, h : h + 1]).astype(np.float16),
                "bq": np.ascontiguousarray(
                    bq[h * HD : (h + 1) * HD].reshape(HD, 1)
                ).astype(np.float32),
                "bk": np.ascontiguousarray(
                    bk[h * HD : (h + 1) * HD].reshape(HD, 1)
                ).astype(np.float32),
            }
        )

    nc = _get_nc()
    res = run_bass_kernel_spmd(nc, in_maps, core_ids=list(range(H)))
    return np.stack([res.results[h]["out"] for h in range(H)], axis=2)


# revision 17
# speedup vs baseline: 8086.5078x; 8086.5078x over previous
"""Trainium2 Bass kernel for nn_MultiHeadDistanceLayer (sparse_attention).

Math: for each (head h, batch b) the reference collapses to
    S[m] = sum_k attn[k-m, k] * w[k],   w[k] = sigmoid(x @ Wv)[L-1-k, h]
(weighted superdiagonal sums of the attention matrix), followed by a
window-3 same-padded average pool over m (padding excluded from the
divisor):  out[b, m, h] = (S[m-1] + S[m] + S[m+1]) / cnt[m].

Sharding: 8 heads -> 8 NeuronCores; each core computes its head for both
batches.  Per (h, b) the kernel runs flash-style over 16 row blocks of 128
queries: scores via PE (K=HD=32), exp on ACT (with free row-sum accum ->
softmax denominators d), gating by w on DVE, then a *skewed* DMA write of
the probability block to a DRAM scratch so that superdiagonal m lands at
column m of every row.  A plain strided read back + a [1/d]-weighted
ones-matmul on PE reduces partitions, PSUM-accumulating S[m] across
blocks.  The window-3 pool runs on-device in 512-chunks pipelined behind
the S accumulation; host only slices inputs / stacks the per-core (B, L)
outputs into (B, L, H).
"""

import contextlib

import numpy as np

import concourse.bacc as bacc
import concourse.bass as bass
import concourse.tile as tile
from concourse import mybir
from concourse.tile import add_dep_helper

B, L, D, H, HD, WIN = 2, 2048, 128, 8, 32, 3
NBLK = L // 128           # 16 row blocks per (h, b)
W = L + 128               # scratch row width (elements)
SCALE = float(HD) ** -0.5

FP16 = mybir.dt.float16
FP32 = mybir.dt.float32

DEFAULT_OPTS = dict(
    ew_bufs=6,
    rt_bufs=4,
    write_gpsimd=False,  # scratch writes via SWDGE (Pool) instead of HWDGE
    zero_bias=False,     # biases known to be zero -> plain copy instead of add
    skip_wmul=False,
    skip_write=False,
    skip_read=False,
    skip_reduce=False,
    skip_post=False,
    kt_act=True,         # prep K-copies on ACT (idle during prep)
    prep1_late=None,     # emit pair-1 prep after this many pair-0 blocks (None=upfront)
    reduce_delay=4,
    memset_dve=True,     # ew tail memset on DVE instead of Pool
)


def build_nc(repeat=1, **opts_kw):
    opts = dict(DEFAULT_OPTS, **opts_kw)
    nc = bacc.Bacc("TRN2", target_bir_lowering=False, debug=False)

    xpT = nc.dram_tensor("xpT", [B, D, L], FP16, kind="ExternalInput")
    xrevT = nc.dram_tensor("xrevT", [B, D, L], FP16, kind="ExternalInput")
    wq = nc.dram_tensor("wq", [D, HD], FP16, kind="ExternalInput")
    wk = nc.dram_tensor("wk", [D, HD], FP16, kind="ExternalInput")
    wv = nc.dram_tensor("wv", [D, 1], FP16, kind="ExternalInput")
    bqk = nc.dram_tensor("bqk", [2 * HD, 1], FP32, kind="ExternalInput")
    out = nc.dram_tensor("out", [B, L], FP32, kind="ExternalOutput")
    # one flat scratch region per (h, b) pair; row i of the logical [L, W]
    # grid holds the skew-shifted probability row i
    scr = [
        nc.dram_tensor(f"scr{b}", [L * W], FP16, kind="Internal") for b in range(B)
    ]

    with tile.TileContext(nc) as tc:
        with contextlib.ExitStack() as ctx:
            singles = ctx.enter_context(tc.tile_pool(name="singles", bufs=1))
            small = ctx.enter_context(tc.tile_pool(name="small", bufs=4))
            ew_pool = ctx.enter_context(tc.tile_pool(name="ew", bufs=opts["ew_bufs"]))
            rt_pool = ctx.enter_context(tc.tile_pool(name="rt", bufs=opts["rt_bufs"]))
            ps_pool = ctx.enter_context(tc.tile_pool(name="ps", bufs=2, space="PSUM"))
            s_pool = ctx.enter_context(tc.tile_pool(name="spsum", bufs=1, space="PSUM"))
            post_pool = ctx.enter_context(tc.tile_pool(name="post", bufs=4))

            # ---- constants (one-time) ----
            # ones row lives at partition 64 to match pv's col-group placement
            ones_row = singles.tile([97, 128], FP16)
            nc.vector.memset(ones_row, 1.0)
            cntr = singles.tile([1, L], FP32)
            nc.vector.memset(cntr, 1.0 / 3.0)
            nc.vector.memset(cntr[0:1, 0:1], 0.5)
            nc.vector.memset(cntr[0:1, L - 1 : L], 0.5)
            # preload ACT table sets (sigmoid first, exp second) while DMAs run
            warm = singles.tile([1, 8], FP32)
            nc.vector.memset(warm, 0.0)
            warm2 = singles.tile([1, 8], FP32)
            nc.scalar.activation(out=warm2, in_=warm,
                                 func=mybir.ActivationFunctionType.Sigmoid)

            prev_rd = {}

            def emit_prep_v(weights):
                """vrev rows: partition 64 = pair 0, partition 96 = pair 1."""
                _, _, _, wv_sb = weights
                xr = {}
                for b in range(B):
                    xr[b] = singles.tile([D, L], FP16, tag=f"xrevT{b}", name=f"xr{b}")
                    nc.sync.dma_start(out=xr[b][:, 0:1024], in_=xrevT[b, :, 0:1024])
                    nc.sync.dma_start(out=xr[b][:, 1024:L], in_=xrevT[b, :, 1024:L])
                vrev = singles.tile([97, L], FP16, tag="vrev")
                for half in range(2):
                    c0 = half * 1024
                    pv = ps_pool.tile([128, 1024], FP32, tag="ps")
                    for j in range(2):
                        for b in range(B):
                            nc.tensor.matmul(
                                out=pv[64 + 32 * b : 65 + 32 * b,
                                       j * 512 : (j + 1) * 512],
                                lhsT=wv_sb[:, :],
                                rhs=xr[b][:, c0 + j * 512 : c0 + (j + 1) * 512],
                                start=True, stop=True,
                                tile_position=(0, 64 + 32 * b),
                            )
                    for b in range(B):
                        nc.scalar.activation(
                            out=vrev[64 + 32 * b : 65 + 32 * b, c0 : c0 + 1024],
                            in_=pv[64 + 32 * b : 65 + 32 * b, :],
                            func=mybir.ActivationFunctionType.Sigmoid,
                        )
                return vrev

            def emit_prep(b, weights, vrev):
                """QT/KT [32, L] fp16, w_bcast [128, L] fp16 for pair b."""
                bqk_sb, wq_sb, wk_sb, wv_sb = weights
                xp_t = singles.tile([D, L], FP16, tag=f"xpT{b}")
                nc.sync.dma_start(out=xp_t[:, 0:1024], in_=xpT[b, :, 0:1024])
                nc.sync.dma_start(out=xp_t[:, 1024:L], in_=xpT[b, :, 1024:L])

                qt = singles.tile([HD, L], FP16, tag=f"QT{b}")
                kt = singles.tile([HD, L], FP16, tag=f"KT{b}")
                # packed prep psum: Q at partitions [0:32), K at [32:64);
                # copies emitted per 512-chunk so the psum slot frees early
                for half in range(2):
                    c0 = half * 1024
                    pqkv = ps_pool.tile([128, 1024], FP32, tag="ps")
                    for j in range(2):
                        cs = slice(c0 + j * 512, c0 + (j + 1) * 512)
                        js = slice(j * 512, (j + 1) * 512)
                        nc.tensor.matmul(
                            out=pqkv[0:HD, js], lhsT=wq_sb[:, :],
                            rhs=xp_t[:, cs], start=True, stop=True,
                        )
                        nc.tensor.matmul(
                            out=pqkv[HD : 2 * HD, js], lhsT=wk_sb[:, :],
                            rhs=xp_t[:, cs], start=True, stop=True,
                        )
                        if opts["zero_bias"]:
                            nc.vector.tensor_copy(
                                out=qt[:, cs], in_=pqkv[0:HD, js]
                            )
                            if opts["kt_act"]:
                                nc.scalar.copy(
                                    out=kt[:, cs], in_=pqkv[HD : 2 * HD, js]
                                )
                            else:
                                nc.vector.tensor_copy(
                                    out=kt[:, cs], in_=pqkv[HD : 2 * HD, js]
                                )
                        else:
                            nc.vector.tensor_scalar_add(
                                out=qt[:, cs],
                                in0=pqkv[0:HD, js],
                                scalar1=bqk_sb[0:HD],
                            )
                            if opts["kt_act"]:
                                nc.scalar.add(
                                    out=kt[:, cs],
                                    in_=pqkv[HD : 2 * HD, js],
                                    add=bqk_sb[HD : 2 * HD],
                                )
                            else:
                                nc.vector.tensor_scalar_add(
                                    out=kt[:, cs],
                                    in0=pqkv[HD : 2 * HD, js],
                                    scalar1=bqk_sb[HD : 2 * HD],
                                )
                # broadcast vrev across 128 partitions via K=1 matmul
                wb = singles.tile([128, L], FP16, tag=f"WB{b}")
                for half in range(2):
                    c0 = half * 1024
                    pb = ps_pool.tile([128, 1024], FP32, tag="ps")
                    for j in range(2):
                        nc.tensor.matmul(
                            out=pb[:, j * 512 : (j + 1) * 512],
                            lhsT=ones_row[64 + 32 * b : 65 + 32 * b, :],
                            rhs=vrev[64 + 32 * b : 65 + 32 * b,
                                     c0 + j * 512 : c0 + (j + 1) * 512],
                            start=True,
                            stop=True,
                            tile_position=(64 + 32 * b, 0),
                        )
                    nc.vector.tensor_copy(out=wb[:, c0 : c0 + 1024], in_=pb[:, :])
                return qt, kt, wb

            def emit_instance():
                bqk_sb = singles.tile([2 * HD, 1], FP32, tag="bqk_sb")
                nc.sync.dma_start(out=bqk_sb, in_=bqk[:, :])
                wq_sb = singles.tile([D, HD], FP16, tag="wq_sb")
                nc.sync.dma_start(out=wq_sb, in_=wq[:, :])
                wk_sb = singles.tile([D, HD], FP16, tag="wk_sb")
                nc.sync.dma_start(out=wk_sb, in_=wk[:, :])
                wv_sb = singles.tile([D, 1], FP16, tag="wv_sb")
                nc.sync.dma_start(out=wv_sb, in_=wv[:, :])
                weights = (bqk_sb, wq_sb, wk_sb, wv_sb)

                QT, KT, WB = {}, {}, {}
                REDUCE_DELAY = opts["reduce_delay"]

                # all sigmoids first (single table-set load), then exp-table
                # warm-up, then the ACT-free Q/K preps
                vrev_all = emit_prep_v(weights)
                VR = {b: vrev_all for b in range(B)}
                warmx = singles.tile([1, 8], FP32, tag="warmx")
                nc.scalar.activation(out=warmx, in_=vrev_all[64:65, 0:8],
                                     func=mybir.ActivationFunctionType.Exp)

                prep1_late = opts["prep1_late"]
                QT[0], KT[0], WB[0] = emit_prep(0, weights, VR[0])
                if prep1_late is None:
                    QT[1], KT[1], WB[1] = emit_prep(1, weights, VR[1])

                for b in range(B):
                    s_acc = s_pool.tile([1, L], FP32, tag="S")
                    res = post_pool.tile([1, L], FP32, tag="res")
                    pending = []
                    done_chunks = set()

                    def emit_reduce(item, s_acc=s_acc):
                        rr, rt_t, dr16, rlen_r = item
                        for j in range(4):
                            m0 = j * 512
                            wj = min(512, rlen_r - m0)
                            if wj <= 0:
                                continue
                            nc.tensor.matmul(
                                out=s_acc[0:1, m0 : m0 + wj],
                                lhsT=dr16[:, 0:1],
                                rhs=rt_t[:, m0 : m0 + wj],
                                start=(rr == 0),
                                stop=(rr == 15 - 4 * j),
                            )

                    def emit_pool_chunk(j, s_acc=s_acc, res=res):
                        """pooled chunk j: needs S[512j-1 .. 512j+512]."""
                        lo = 512 * j - 1
                        hi = 512 * j + 513
                        sS = post_pool.tile([1, 516], FP32, tag="sS")
                        if lo < 0:
                            nc.vector.memset(sS[0:1, 0:1], 0.0)
                        if hi > L:
                            nc.vector.memset(sS[0:1, 513:514], 0.0)
                        src_lo = max(lo, 0)
                        dst_lo = src_lo - lo
                        src_hi = min(hi, L)
                        # tail chunks copy on ACT (idle there); early ones DVE
                        if j <= 1:
                            nc.scalar.copy(
                                out=sS[0:1, dst_lo : dst_lo + src_hi - src_lo],
                                in_=s_acc[0:1, src_lo:src_hi],
                            )
                        else:
                            nc.vector.tensor_copy(
                                out=sS[0:1, dst_lo : dst_lo + src_hi - src_lo],
                                in_=s_acc[0:1, src_lo:src_hi],
                            )
                        t1 = post_pool.tile([1, 512], FP32, tag="t1")
                        nc.vector.tensor_add(
                            out=t1, in0=sS[0:1, 0:512], in1=sS[0:1, 1:513]
                        )
                        t2 = post_pool.tile([1, 512], FP32, tag="t2")
                        nc.vector.tensor_add(out=t2, in0=t1, in1=sS[0:1, 2:514])
                        nc.vector.tensor_mul(
                            out=res[0:1, 512 * j : 512 * (j + 1)],
                            in0=t2,
                            in1=cntr[0:1, 512 * j : 512 * (j + 1)],
                        )

                    def maybe_pool(r_done, done_chunks=done_chunks,
                                   emit_pool_chunk=emit_pool_chunk):
                        # chunk c of S is final after block 15-4c; pooled
                        # chunk j additionally needs chunk j-1 (block 19-4j)
                        for j in range(3, -1, -1):
                            if j in done_chunks:
                                continue
                            need = 15 - 4 * (j - 1) if j >= 1 else 15
                            if r_done >= need:
                                done_chunks.add(j)
                                emit_pool_chunk(j)

                    for r in range(NBLK):
                        if b == 0 and prep1_late is not None and r == prep1_late:
                            QT[1], KT[1], WB[1] = emit_prep(1, weights, VR[1])
                        i0 = r * 128
                        ew = ew_pool.tile([128, W], FP16, tag="ew")
                        if opts["memset_dve"]:
                            nc.vector.memset(ew[:, L:W], 0.0)
                        else:
                            nc.gpsimd.memset(ew[:, L:W], 0.0)
                        dcol = []
                        for half in range(2):
                            c0 = half * 1024
                            ps = ps_pool.tile([128, 1024], FP32, tag="ps")
                            for j in range(2):
                                nc.tensor.matmul(
                                    out=ps[:, j * 512 : (j + 1) * 512],
                                    lhsT=QT[b][:, i0 : i0 + 128],
                                    rhs=KT[b][:, c0 + j * 512 : c0 + (j + 1) * 512],
                                    start=True,
                                    stop=True,
                                )
                            dc = small.tile([128, 1], FP32, tag="dc")
                            nc.scalar.activation(
                                out=ew[:, c0 : c0 + 1024],
                                in_=ps[:, :],
                                func=mybir.ActivationFunctionType.Exp,
                                scale=SCALE,
                                accum_out=dc,
                            )
                            dcol.append(dc)
                        dsum = small.tile([128, 1], FP32, tag="ds")
                        nc.vector.tensor_add(out=dsum, in0=dcol[0], in1=dcol[1])
                        drecip16 = small.tile([128, 1], FP16, tag="dr16")
                        with nc.allow_low_precision("1/d in fp16; error washes out"):
                            nc.vector.reciprocal(out=drecip16, in_=dsum)

                        # gate by w (columns i0..L of this block: upper triangle)
                        if not opts["skip_wmul"]:
                            nc.vector.tensor_mul(
                                out=ew[:, i0:L],
                                in0=ew[:, i0:L],
                                in1=WB[b][:, i0:L],
                            )

                        # skewed write: scratch[i0+p, c - i0 - p] = ew[p, c]
                        # flat dst = i0*W + p*(W-1) + (c - i0)
                        wlen = (L + 127) - i0
                        dst = bass.AP(
                            tensor=scr[b],
                            offset=i0 * W,
                            ap=[[W - 1, 128], [1, wlen]],
                        )
                        wr = None
                        if not opts["skip_write"]:
                            weng = nc.gpsimd if opts["write_gpsimd"] else nc.sync
                            wr = weng.dma_start(out=dst, in_=ew[:, i0 : L + 127])
                            if (b, r) in prev_rd:
                                add_dep_helper(
                                    wr.ins, prev_rd[(b, r)], True,
                                    "scr WAR vs prev rep",
                                )

                        # plain read back: rows i0..i0+128, cols 0..L-i0
                        rlen = L - i0
                        src = bass.AP(
                            tensor=scr[b],
                            offset=i0 * W,
                            ap=[[W, 128], [1, rlen]],
                        )
                        rt = rt_pool.tile([128, L], FP16, tag="rt")
                        if not opts["skip_read"]:
                            rd = nc.sync.dma_start(out=rt[:, 0:rlen], in_=src)
                            if wr is not None:
                                add_dep_helper(rd.ins, wr.ins, True, "scratch RAW")
                            prev_rd[(b, r)] = rd.ins

                        # S[m] += sum_p (1/d[i0+p]) * rt[p, m] -- emitted a few
                        # blocks late so PE isn't head-of-line blocked on the
                        # scratch round-trip
                        if opts["skip_reduce"] or opts["skip_read"]:
                            continue
                        pending.append((r, rt, drecip16, rlen))
                        if len(pending) > REDUCE_DELAY:
                            item = pending.pop(0)
                            emit_reduce(item)
                            maybe_pool(item[0])

                    if not (opts["skip_reduce"] or opts["skip_read"]):
                        while pending:
                            item = pending.pop(0)
                            emit_reduce(item)
                            maybe_pool(item[0])

                    if opts["skip_post"] or opts["skip_reduce"] or opts["skip_read"]:
                        continue
                    nc.sync.dma_start(out=out[b, :], in_=res[0:1, :])

            for _rep in range(repeat):
                emit_instance()

    nc.finalize()
    return nc


_RUNNERS = {}


def _get_runner(repeat=1, **opts_kw):
    key = (repeat, tuple(sorted(opts_kw.items())))
    if key in _RUNNERS:
        return _RUNNERS[key]
    import jax
    from jax.experimental.shard_map import shard_map
    from jax.sharding import Mesh, PartitionSpec

    from concourse import bass2jax

    nc = build_nc(repeat, **opts_kw)
    bass2jax.install_neuronx_cc_hook()

    partition_name = nc.partition_id_tensor.name if nc.partition_id_tensor else None
    in_names, out_names, out_avals = [], [], []
    for alloc in nc.m.functions[0].allocations:
        if not isinstance(alloc, mybir.MemoryLocationSet):
            continue
        name = alloc.memorylocations[0].name
        if alloc.kind == "ExternalInput":
            if name != partition_name:
                in_names.append(name)
        elif alloc.kind == "ExternalOutput":
            out_names.append(name)
            out_avals.append(
                jax.core.ShapedArray(
                    tuple(alloc.tensor_shape), mybir.dt.np(alloc.dtype)
                )
            )
    n_params = len(in_names)
    n_outs = len(out_names)
    all_in = list(in_names) + list(out_names)
    if partition_name is not None:
        all_in.append(partition_name)

    def _body(*args):
        operands = list(args)
        if partition_name is not None:
            operands.append(bass2jax.partition_id_tensor())
        outs = bass2jax._bass_exec_p.bind(
            *operands,
            out_avals=tuple(out_avals),
            in_names=tuple(all_in),
            out_names=tuple(out_names),
            lowering_input_output_aliases=(),
            sim_require_finite=True,
            sim_require_nnan=True,
            nc=nc,
        )
        return tuple(outs)

    devices = jax.devices()[:H]
    mesh = Mesh(np.asarray(devices), ("core",))
    sharded = jax.jit(
        shard_map(
            _body,
            mesh=mesh,
            in_specs=(PartitionSpec("core"),) * (n_params + n_outs),
            out_specs=(PartitionSpec("core"),) * n_outs,
            check_rep=False,
        ),
        donate_argnums=tuple(range(n_params, n_params + n_outs)),
        keep_unused=True,
    )
    runner = (sharded, in_names, out_names, out_avals)
    _RUNNERS[key] = runner
    return runner


def _prep_in_maps(x, pe, Wq, bq, Wk, bk, Wv):
    x = np.asarray(x, np.float32)
    pe = np.asarray(pe, np.float32)
    Wq = np.asarray(Wq, np.float32)
    bq = np.asarray(bq, np.float32)
    Wk = np.asarray(Wk, np.float32)
    bk = np.asarray(bk, np.float32)
    Wv = np.asarray(Wv, np.float32)

    xp = x + pe[None, :, :]
    xpT = np.ascontiguousarray(xp.transpose(0, 2, 1)).astype(np.float16)
    xrevT = np.ascontiguousarray(x[:, ::-1, :].transpose(0, 2, 1)).astype(np.float16)

    in_maps = []
    for h in range(H):
        hs = slice(h * HD, (h + 1) * HD)
        bqk = np.concatenate([bq[hs], bk[hs]]).reshape(2 * HD, 1)
        in_maps.append(
            {
                "xpT": xpT,
                "xrevT": xrevT,
                "wq": np.ascontiguousarray(Wq[:, hs]).astype(np.float16),
                "wk": np.ascontiguousarray(Wk[:, hs]).astype(np.float16),
                "wv": np.ascontiguousarray(Wv[:, h : h + 1]).astype(np.float16),
                "bqk": np.ascontiguousarray(bqk).astype(np.float32),
            }
        )
    return in_maps


def run(in_maps, repeat=1, **opts_kw):
    sharded, in_names, out_names, out_avals = _get_runner(repeat, **opts_kw)
    concat_in = [
        np.concatenate([np.asarray(in_maps[c][n]) for c in range(H)], axis=0)
        for n in in_names
    ]
    concat_zeros = [
        np.zeros((H * a.shape[0], *a.shape[1:]), a.dtype) for a in out_avals
    ]
    out_arrs = sharded(*concat_in, *concat_zeros)
    return [
        {
            n: np.asarray(out_arrs[i]).reshape(H, *out_avals[i].shape)[c]
            for i, n in enumerate(out_names)
        }
        for c in range(H)
    ]


def kernel(x, pe, Wq, bq, Wk, bk, Wv):
    in_maps = _prep_in_maps(x, pe, Wq, bq, Wk, bk, Wv)
    zb = not (np.any(np.asarray(bq)) or np.any(np.asarray(bk)))
    results = run(in_maps, repeat=1, zero_bias=bool(zb))
    return np.stack([results[h]["out"] for h in range(H)], axis=2)
